# revision 30
# baseline (speedup 1.0000x reference)
"""Trainium2 Bass kernel for nn_AttentionModule_16398185136487.

Math (the reference reduces to this — its trailing softmax is over a size-1
axis, i.e. ones):
  out = concat([x34, a_x4, x43, b_x3], axis=1)            # (8, 512, 32, 32)
  block(qs, ks, v) = gate(qs, ks) * (w128@wv @ x_v + w128@bv) + b128
  gate(qs, ks)[b, hw] = softmax_hw( (1/8) sum_{kb} max_{khw}
                                    (Q_qs[b,hw] . K_ks[kb,khw]) / 16 )

Sharding: core j owns batch image j (its 1024 query pixels for both the x4
and x3 streams) — the per-image softmax is then fully core-local; no
collectives.

The 1x1 convs are computed host-side (fp32 exact); the device does the
compute-bound part: per core a (2048 q) x (16384 key) fp8 DoubleRow score
GEMM (512 matmuls of [256->DR]x[128q]x[512k], 217 ns each at full clock)
plus the per-image key-max reduction, which is PSUM-port-bound: every
score element must cross ScalarE (1.2 GHz) or VectorE (0.96 GHz), the only
engines with PSUM read ports.

Consumer lanes (per [128q, 1024k] score tile, split by (img+qh) parity so
both engines run every iteration; assignment uniform in qs per gate so the
LSE offset cancels in that gate's softmax):
  - ScalarE lane ('A'): one-pass exp(BETA*(s-C)) activation with
    accum_out -> per-row exp-sum (LSE ~ max for BETA=12); the Schraudolph
    identity (int32 bit pattern ~ 2^23*(log2+127)) folds the log into the
    per-stream sums.
  - VectorE lane ('C'): exact reduce_max over the 2-bank PSUM tile.
Schedule: image-major (16 q-tiles per image; QS_SEQ alternates q-halves so
the lanes alternate every tile), 4 shared 2-bank PSUM score slots.  K
streams image-by-image on the sync+gpsimd queues (~5.7us/image, vs
~10.6us/image consumption) interleaved with qs-ordered q8 chunks; the PE
then runs long uninterrupted matmul runs and holds its full 2.4 GHz
p-state (216 ns per 512-col fp8-DR matmul).  The four gate softmaxes are
emitted one gate at a time: aa/ba mid-stream (fully overlapped), bb after
the qh1 tiles of the interleaved last two images, ab at the end on a
shortened chain (scalar reciprocal needs no transpose hop; PE broadcasts).
"""

import numpy as np
import ml_dtypes

B = 8
C = 256
HW = 1024          # 32*32
BHW = B * HW       # 8192
NCORES = 8

BETA = 12.0        # LSE sharpness in raw-score units
CBIAS = 9.0        # exp bias: exp(BETA*(s - CBIAS)) stays in fp32 range

_CACHE = {}


def _ref_gmul_bias(in0, in1, c0, c1, c2):
    return (in0.astype(np.float32) * in1 * c1 + c0).astype(np.float32)


def _get_custom_ops():
    """Register the GMUL_BIAS custom DVE microcode op:
      GMUL_BIAS: out = in0 * in1 * s1 + s0     (s0, s1 per-partition APs)
    """
    if "ops" in _CACHE:
        return _CACHE["ops"]
    import concourse.dve_ops as dve_ops
    from concourse.dve_ops import DveOp
    from concourse.dve_spec import Spec, Src0, Src1, C0, C1, lower
    from concourse.dve_uop import DveOpSpec

    def register(name, spec):
        for op in dve_ops.OPS:
            if op.name == name:
                return op
        shas = {}
        for ver in ("v3", "v4"):
            shas[ver] = DveOpSpec(name=name, opcode=1,
                                  uops=lower(spec, ver=ver),
                                  rd1_en=True).sha(ver)
        op = DveOp(name, spec, subdim=False, uops_sha=shas)
        dve_ops.OPS.append(op)
        dve_ops.CUSTOM_DVE_SPECS[op.name] = op.spec
        dve_ops._SUB_OPCODE_FOR_NAME[op.name] = (
            dve_ops._CUSTOM_DVE_ROW_BASE + len(dve_ops.OPS) - 1)
        assert max(dve_ops._SUB_OPCODE_FOR_NAME.values()) < 0x20
        return op

    gmul = register("GMUL_BIAS",
                    Spec(body=Src0 * Src1 * C1 + C0,
                         reference=_ref_gmul_bias))
    _CACHE["ops"] = (gmul,)
    return _CACHE["ops"]


# method('A'=ScalarE LSE | 'C'=DVE exact max) per (img, qhalf); uniform
# across the 8 q-tiles of each gate so the -BETA*CBIAS offset of A-images
# cancels in that gate's softmax.  Parity assignment feeds BOTH engines
# every iteration.
def _method(img, qs):
    if (img, qs) in FLIPS:
        return 'F'
    return 'A' if (img + qs // 8) % 2 == 0 else 'C'


# ScalarE's accum-read makes its per-tile cost ~1.15x DVE's: flip 10
# scattered A-tiles to exact-max ('F').  Each flip sits at a distinct
# (gate, qs-column); the flipped max goes to mfxg and the fixup adds it
# plus the deterministic A-term bias -(CBIAS - 127*ln2/BETA) that the
# column now lacks (per-column constant, exact).  The flipped m_all
# columns are zeroed so the A-box bit-pattern sums see exactly 0.
FLIPS = frozenset()   # rebalance abandoned: the extra DVE ops cost more
                      # than the ScalarE relief (measured 258us vs 193us)


def _build_nc():
    from contextlib import ExitStack

    import concourse.bass as bass
    import concourse.mybir as mybir
    import concourse.tile as tile
    from concourse import bacc
    from concourse.masks import make_identity

    f32 = mybir.dt.float32
    bf16 = mybir.dt.bfloat16
    fp8 = mybir.dt.float8e4
    i32 = mybir.dt.int32
    AX = mybir.AxisListType.X
    AXY = mybir.AxisListType.XY
    Exp = mybir.ActivationFunctionType.Exp
    DR = mybir.MatmulPerfMode.DoubleRow

    (gmul,) = _get_custom_ops()
    nc = bacc.Bacc("TRN2", target_bir_lowering=False, debug=False,
                   enable_asserts=False, num_devices=NCORES)

    # DRAM I/O (per core); features precomputed host-side
    q8_ap = nc.dram_tensor("q8", (128, 2 * 2 * HW), fp8,
                           kind="ExternalInput").ap()
    ka_ap = nc.dram_tensor("ka8", (128, 2 * BHW), fp8,
                           kind="ExternalInput").ap()
    kb_ap = nc.dram_tensor("kb8", (128, 2 * BHW), fp8,
                           kind="ExternalInput").ap()
    va_ap = nc.dram_tensor("va", (128, HW), f32, kind="ExternalInput").ap()
    vb_ap = nc.dram_tensor("vb", (128, HW), f32, kind="ExternalInput").ap()
    b128_ap = nc.dram_tensor("b128", (128, 1), f32, kind="ExternalInput").ap()
    out_ap = nc.dram_tensor("out", (512, HW), f32, kind="ExternalOutput").ap()

    SCALE_EFF = (1.0 / 16.0) / 8.0  # /sqrt(C), /8 mean

    with tile.TileContext(nc) as tc:
        with ExitStack() as ctx:
            const = ctx.enter_context(tc.tile_pool(name="const", bufs=1))
            ps_a = ctx.enter_context(
                tc.tile_pool(name="psa", bufs=4, space="PSUM"))
            scr = ctx.enter_context(tc.tile_pool(name="scr", bufs=3))
            gp = ctx.enter_context(tc.tile_pool(name="gp", bufs=2))
            fin = ctx.enter_context(tc.tile_pool(name="fin", bufs=2))

            # ---- resident feature tiles, streamed image-by-image ----
            q8 = const.tile([128, 2 * 2 * HW], fp8, tag="q8", name="q8")
            ka8 = const.tile([128, 2 * BHW], fp8, tag="ka8", name="ka8")
            kb8 = const.tile([128, 2 * BHW], fp8, tag="kb8", name="kb8")
            q3 = q8.rearrange("p (s n) -> p s n", s=2)
            ka3 = ka8.rearrange("p (s n) -> p s n", s=2)
            kb3 = kb8.rearrange("p (s n) -> p s n", s=2)
            va_sb = const.tile([128, HW], f32, tag="va", name="va")
            vb_sb = const.tile([128, HW], f32, tag="vb", name="vb")
            b128_sb = const.tile([128, 1], f32, tag="b128", name="b128")

            def dma_kimg(img, split=False):
                # one key image (both ci chunks): ci0 -> sync, ci1 -> gpsimd
                k8, kap = (ka8, ka_ap) if img < 8 else (kb8, kb_ap)
                n2 = img % 8
                for ci, eng in ((0, nc.sync), (1, nc.gpsimd)):
                    lo = ci * BHW + n2 * 1024
                    if split:       # halve the first image's latency
                        for h in range(2):
                            eng.dma_start(k8[:, lo + h * 512:lo + h * 512 + 512],
                                          kap[:, lo + h * 512:lo + h * 512 + 512])
                    else:
                        eng.dma_start(k8[:, lo:lo + 1024], kap[:, lo:lo + 1024])

            # All input DMA issues live on the sync + gpsimd queues
            # (ScalarE/VectorE are the compute bottleneck; issuing costs
            # ~0.6us of queue time each).  q8 chunks (ci0->sync,
            # ci1->gpsimd) are interleaved with the K image stream in
            # qs-consumption order (QS_SEQ alternates q halves).
            def dma_q8(c0):
                nc.sync.dma_start(q8[:, c0:c0 + 512], q8_ap[:, c0:c0 + 512])
                nc.gpsimd.dma_start(q8[:, 2048 + c0:2048 + c0 + 512],
                                    q8_ap[:, 2048 + c0:2048 + c0 + 512])

            # the first q8 chunk pair rides the otherwise-idle scalar queue
            # so it streams in parallel with K image 0
            nc.scalar.dma_start(q8[:, 0:512], q8_ap[:, 0:512])
            nc.scalar.dma_start(q8[:, 2048:2560], q8_ap[:, 2048:2560])
            dma_kimg(0, split=True)
            dma_q8(1024)
            dma_kimg(1)
            dma_q8(512)
            dma_kimg(2)
            dma_q8(1536)
            for img in range(3, 16):
                dma_kimg(img)
            nc.gpsimd.dma_start(b128_sb[:], b128_ap[:, :])
            # V is first needed by the mid-stream gate emissions (~70% in)
            nc.sync.dma_start(va_sb[:], va_ap[:, :])
            nc.gpsimd.dma_start(vb_sb[:], vb_ap[:, :])

            ones_row = const.tile([1, 128], f32, tag="ones_row", name="ones_row")
            nc.vector.memset(ones_row[:], 1.0)
            ones_row_bf = const.tile([1, 128], bf16, tag="ones_bf",
                                     name="ones_bf")
            nc.vector.memset(ones_row_bf[:], 1.0)
            ones_col = const.tile([128, 1], f32, tag="ones_col", name="ones_col")
            nc.vector.memset(ones_col[:], 1.0)
            nbias = const.tile([128, 1], f32, tag="nbias", name="nbias")
            nc.vector.memset(nbias[:], -BETA * CBIAS)
            ident = const.tile([128, 128], f32, tag="ident", name="ident")
            make_identity(nc, ident[:])

            # per-(img, q-tile) reductions: col = img*16 + qs.
            m_all = const.tile([128, 256], f32, tag="m_all", name="m_all")
            Mka = const.tile([128, 16], f32, tag="Mka", name="Mka")  # (aa, ba)
            Mkb = const.tile([128, 16], f32, tag="Mkb", name="Mkb")  # (ab, bb)
            # flipped-tile maxima (col = gate*8 + qs%8) + their per-column
            # A-term bias corrections
            if FLIPS:
                mfxg = const.tile([128, 32], f32, tag="mfxg", name="mfxg")
                nc.vector.memset(mfxg[:], 0.0)
                corrg = const.tile([128, 32], f32, tag="corrg", name="corrg")
                nc.vector.memset(corrg[:], 0.0)
                CORR = float(-(CBIAS - 127.0 * np.log(2.0) / BETA))
                for img, qs in sorted(FLIPS):
                    g = (0 if img < 8 else 2) + qs // 8
                    nc.vector.memset(
                        corrg[:, g * 8 + qs % 8:g * 8 + qs % 8 + 1], CORR)
                    nc.vector.memset(
                        m_all[:, img * 16 + qs:img * 16 + qs + 1], 0.0)

            def kimg3(img):
                return ka3 if img < 8 else kb3

            def score_mms_into(dst, img, qs):
                # two 512-col DR matmuls for one (q-tile, image) into dst
                k3 = kimg3(img)
                n2 = img % 8
                qcol = qs * 128
                for half in range(2):
                    kcol = n2 * HW + half * 512
                    nc.tensor.matmul(
                        dst[:, half * 512:(half + 1) * 512],
                        q3[:, :, qcol:qcol + 128],
                        k3[:, :, kcol:kcol + 512],
                        start=True, stop=True, perf_mode=DR)

            # ---- one (q-tile, image) score tile + its consumer ----
            def emit_tile(qs, img):
                t = ps_a.tile([128, 1024], f32, tag="psa", name="sc")
                score_mms_into(t, img, qs)
                col = img * 16 + qs
                meth = _method(img, qs)
                if meth == 'A':
                    esc = scr.tile([128, 1024], bf16, tag="esc",
                                   name="esc", bufs=3)
                    nc.scalar.activation(
                        esc[:], t[:, 0:1024], Exp, bias=nbias[:],
                        scale=BETA, accum_out=m_all[:, col:col + 1])
                elif meth == 'F':
                    g = (0 if img < 8 else 2) + qs // 8
                    gc = g * 8 + qs % 8
                    nc.vector.reduce_max(
                        mfxg[:, gc:gc + 1], t[:, 0:1024], axis=AX)
                else:
                    nc.vector.reduce_max(
                        m_all[:, col:col + 1], t[:, 0:1024], axis=AX)

            # ---- per-stream fixup: fold the A-col exp-sums into the sums ----
            # img = x*4 + y2*2 + y1: A-imgs for qh=0 are even (y1=0), for
            # qh=1 odd (y1=1); the stream picks x (a: 0..1, b: 2..3).
            mperm = m_all.rearrange("p (x y2 y1 q) -> p q x y2 y1",
                                    x=4, y2=2, y1=2)

            def emit_fixup_half(Mdst, lo, qh):
                # one gate's 8 columns of Mdst: Schraudolph-sum the A
                # columns (int bit patterns, materialized as f32 ints by
                # the reduce) + f32-sum the C (exact max) columns.
                xb = 0 if lo == 0 else 2
                sl = slice(qh * 8, qh * 8 + 8)
                a_y1 = qh           # qh0: A = even imgs; qh1: A = odd
                T1 = gp.tile([128, 8], f32, tag=f"T1{lo}{qh}", name="T1")
                T2 = gp.tile([128, 8], f32, tag=f"T2{lo}{qh}", name="T2")
                nc.vector.reduce_sum(
                    T2[:], mperm[:, sl, xb:xb + 2, 0:2, a_y1].bitcast(i32),
                    axis=AXY)
                nc.vector.reduce_sum(
                    T1[:], mperm[:, sl, xb:xb + 2, 0:2, 1 - a_y1], axis=AXY)
                nc.vector.scalar_tensor_tensor(
                    Mdst[:, sl], T2[:],
                    float(np.log(2.0) / (BETA * 2.0 ** 23)), T1[:],
                    op0=mybir.AluOpType.mult, op1=mybir.AluOpType.add)
                if FLIPS:
                    g = (0 if lo == 0 else 2) + qh
                    sl8 = slice(g * 8, g * 8 + 8)
                    nc.vector.tensor_tensor(
                        Mdst[:, sl], Mdst[:, sl], mfxg[:, sl8],
                        op=mybir.AluOpType.add)
                    nc.vector.tensor_tensor(
                        Mdst[:, sl], Mdst[:, sl], corrg[:, sl8],
                        op=mybir.AluOpType.add)

            # ---- softmax + apply for a single gate ----
            def emit_gate_single(Mhalf, v_sb, blk, tagx, pe_bcast):
                E = gp.tile([128, 8], f32, tag=f"E{tagx}", name="E")
                nc.scalar.activation(E[:], Mhalf, Exp, bias=0.0,
                                     scale=SCALE_EFF)
                # the E-transpose -> grow-DMA path (1.8us DMA latency)
                # is independent of the reciprocal chain: issue it FIRST
                # so the latency overlaps the recip/broadcast hops
                tpe = ps_a.tile([128, 1024], f32, tag="psa", name="tpe")
                nc.tensor.transpose(tpe[0:8, 0:128], E[:], ident[:])
                et = gp.tile([8, 128], bf16, tag=f"et{tagx}", name="et")
                nc.vector.tensor_copy(et[:], tpe[0:8, 0:128])
                grow = gp.tile([1, 1024], bf16, tag=f"grow{tagx}", name="grow")
                nc.sync.dma_start(grow.rearrange("a (t p) -> a t p", t=8),
                                  et[:])
                sr = gp.tile([128, 1], f32, tag=f"sr{tagx}", name="sr")
                nc.vector.reduce_sum(sr[:], E[:], axis=AX)
                sum_ps = ps_a.tile([128, 1024], f32, tag="psa", name="sum_ps")
                nc.tensor.matmul(sum_ps[0:1, 0:1], sr[:], ones_col[:],
                                 start=True, stop=True)
                rec = gp.tile([1, 1], f32, tag=f"rec{tagx}", name="rec")
                nc.vector.reciprocal(rec[:], sum_ps[0:1, 0:1])
                bc = ps_a.tile([128, 1024], f32, tag="psa", name="bc")
                nc.tensor.matmul(bc[:, 0:1], ones_row[:], rec[:],
                                 start=True, stop=True)
                rsb = gp.tile([128, 1], f32, tag=f"rsb{tagx}", name="rsb")
                nc.vector.tensor_copy(rsb[:], bc[:, 0:1])
                out_t = fin.tile([128, HW], f32, tag="out_t", name="out_t")
                for half in range(2):
                    sl_g = grow[0:1, half * 512:(half + 1) * 512]
                    if pe_bcast:
                        # PSUM free at the tail: PE broadcast is faster
                        gbp = ps_a.tile([128, 1024], f32, tag="psa",
                                        name="gbp")
                        nc.tensor.matmul(gbp[:, 0:512], ones_row_bf[:],
                                         sl_g, start=True, stop=True)
                        gb = gbp[:, 0:512]
                    else:
                        gbt = gp.tile([128, 512], bf16, tag=f"gb{tagx}",
                                      name="gb", bufs=2)
                        nc.gpsimd.partition_broadcast(gbt[:, :], sl_g)
                        gb = gbt[:, :]
                    nc.vector._custom_dve(
                        gmul, out=out_t[:, half * 512:(half + 1) * 512],
                        in0=gb, in1=v_sb[:, half * 512:(half + 1) * 512],
                        s0=b128_sb[:], s1=rsb[:, 0:1])
                    eng = nc.sync if half == 0 else nc.gpsimd
                    eng.dma_start(
                        out_ap[blk * 128:(blk + 1) * 128,
                               half * 512:(half + 1) * 512],
                        out_t[:, half * 512:(half + 1) * 512])

            # ---- main schedule: image-major (K streams image-by-image at
            # ~5.7us/image across two queues; consuming 16 q-tiles of one
            # image takes ~11us, so the stream stays ahead).  QS_SEQ
            # alternates q-halves, so the A/C consumer lanes alternate
            # within every image. ----
            QS_SEQ = [0, 8, 1, 9, 2, 10, 3, 11, 4, 12, 5, 13, 6, 14, 7, 15]
            for img in range(8):               # x4-stream key images
                for qs in QS_SEQ:
                    emit_tile(qs, img)
            emit_fixup_half(Mka, 0, 0)
            emit_fixup_half(Mka, 0, 1)
            for img in range(8, 12):           # x3-stream key images 8..11
                for qs in QS_SEQ:
                    emit_tile(qs, img)
            # (aa -> block 1, ba -> block 0); emitted mid-stream so the
            # serial softmax chains overlap the score loop
            emit_gate_single(Mka[:, 0:8], va_sb, 1, "aa", pe_bcast=False)
            emit_gate_single(Mka[:, 8:16], va_sb, 0, "ba", pe_bcast=False)
            for img in range(12, 14):
                for qs in QS_SEQ:
                    emit_tile(qs, img)
            # last two images interleaved, qh1 q-tiles first: their A/C
            # parities are opposite so the lanes alternate, and gate 'bb'
            # overlaps the remaining qh0 tiles
            for qs in range(8, 16):
                emit_tile(qs, 14)
                emit_tile(qs, 15)
            emit_fixup_half(Mkb, 8, 1)
            emit_gate_single(Mkb[:, 8:16], vb_sb, 3, "bb", pe_bcast=False)
            for qs in range(0, 8):
                emit_tile(qs, 14)
                emit_tile(qs, 15)
            emit_fixup_half(Mkb, 8, 0)
            emit_gate_single(Mkb[:, 0:8], vb_sb, 2, "ab", pe_bcast=True)

    nc.compile()
    return nc


def get_nc():
    if "nc" not in _CACHE:
        _CACHE["nc"] = _build_nc()
    return _CACHE["nc"]


def prepare_in_maps(x4, x3, wq, bq, wk, bk, wv, bv, w128, b128):
    """Host-side 1x1 convs (exact fp32) + fp8 quantization + layouts."""
    f8 = ml_dtypes.float8_e4m3
    x4 = np.asarray(x4, np.float32)
    x3 = np.asarray(x3, np.float32)
    X4 = np.ascontiguousarray(x4.transpose(1, 0, 2, 3).reshape(C, BHW))
    X3 = np.ascontiguousarray(x3.transpose(1, 0, 2, 3).reshape(C, BHW))
    wq = np.asarray(wq, np.float32)
    wk = np.asarray(wk, np.float32)
    wv = np.asarray(wv, np.float32)
    w128 = np.asarray(w128, np.float32)
    bqc = np.asarray(bq, np.float32)[:, None]
    bkc = np.asarray(bk, np.float32)[:, None]

    def feat8(w, b, X):
        # (256, N) fp8 feature map -> [128, 2*N] (ci-chunk major)
        f = (w @ X + b).astype(f8)
        N = f.shape[1]
        return np.ascontiguousarray(
            f.reshape(2, 128, N).transpose(1, 0, 2).reshape(128, 2 * N))

    KA = feat8(wk, bkc, X4)                     # (128, 16384) fp8
    KB = feat8(wk, bkc, X3)
    wv128 = w128 @ wv                           # (128, 256)
    bv128 = (w128 @ np.asarray(bv, np.float32))[:, None]
    VA = (wv128 @ X4 + bv128).astype(np.float32)   # (128, 8192)
    VB = (wv128 @ X3 + bv128).astype(np.float32)
    b128r = np.asarray(b128, np.float32).reshape(128, 1)

    in_maps = []
    for j in range(NCORES):
        sl = slice(j * HW, (j + 1) * HW)
        xq = np.concatenate([X4[:, sl], X3[:, sl]], axis=1)   # (256, 2048)
        Q = feat8(wq, bqc, xq)                  # (128, 4096) fp8
        in_maps.append({
            "q8": Q,
            "ka8": KA, "kb8": KB,
            "va": np.ascontiguousarray(VA[:, sl]),
            "vb": np.ascontiguousarray(VB[:, sl]),
            "b128": b128r,
        })
    return in_maps


def kernel(**inputs):
    from concourse.bass_utils import run_bass_kernel_spmd

    nc = get_nc()
    in_maps = prepare_in_maps(**inputs)
    res = run_bass_kernel_spmd(nc, in_maps, core_ids=list(range(NCORES)))
    out = np.stack([res.results[c]["out"].reshape(512, 32, 32)
                    for c in range(NCORES)])
    return np.ascontiguousarray(out.astype(np.float32))
